# revision 1
# baseline (speedup 1.0000x reference)
"""DualAttention Trainium2 Bass kernel (8-core data-parallel).

Contract: kernel(**inputs) takes the FULL inputs of nn_DualAttention
(B=1024, L=199, V=50000, D=Dp=128) and returns the full [1024, 128] f32
output, equal to reference.reference(**inputs).

Strategy (per core, 128 batch rows):
 - only the LAST attention row is ever used by the reference output, so we
   compute q for the last (mean) token only: scores are [128, 200] per core.
 - embeddings + weights cast to bf16 on host; per-core compacted item table
   (unique rows + a zeros row; masked x==0 tokens redirect to the zeros row,
   which is exactly equivalent for the final output).
 - device: indirect-DMA row gathers (token-major) -> HWDGE xbar DMA
   transposes -> feature-major xeT/peT [128, 25600] bf16; masked mean via a
   pairwise-add tree; K^T feature-major + scores via per-batch M=1 matmuls
   packed 8-per-PSUM-bank; entmax bisection (30 iters, batch-major, ACT
   Ln/Exp with per-partition alpha scales); V token-major via stationary-xT
   matmuls; AV with stationary-v tiles -> att^T; PE transpose + relu +
   L2-normalize.
"""
import sys
sys.path.insert(0, '/opt/trn_rl_repo')

import math
import numpy as np
import ml_dtypes

import concourse.bass as bass
import concourse.bacc as bacc
import concourse.mybir as mybir
import concourse.tile as tile
from concourse.bass_utils import run_bass_kernel_spmd

F32 = mybir.dt.float32
BF16 = mybir.dt.bfloat16
I32 = mybir.dt.int32

B, L, V, D = 1024, 199, 50000, 128
P = L + 1                 # 200 tokens (199 items + mean)
NB = 128                  # batches per core
NCORES = 8
NT = 200                  # 128-token gather tiles per table
NCOL = NB * P             # 25600 flat columns (col = 200*b + t)
TBL_ROWS = NCOL + 128     # fixed-size per-core compact table (padded)
N_ITER = 26               # bisection iterations (f32-converged by ~26)
AluOp = mybir.AluOpType
Act = mybir.ActivationFunctionType

_cache = {}
_last_in_maps = None


def _build(ba_const: float):
    nc = bacc.Bacc(None, target_bir_lowering=False, debug=False)

    tbl = nc.declare_dram_parameter("tbl", [TBL_ROWS, D], BF16, isOutput=False)
    ptbl = nc.declare_dram_parameter("ptbl", [P, D], BF16, isOutput=False)
    idxi = nc.declare_dram_parameter("idxi", [128, NT], I32, isOutput=False)
    idxp = nc.declare_dram_parameter("idxp", [128, NT], I32, isOutput=False)
    mb = nc.declare_dram_parameter("mb", [NB, P], F32, isOutput=False)
    wts = {}
    for w in ("wk0", "wk1", "wv0", "wv1", "wq0", "wq1"):
        wts[w] = nc.declare_dram_parameter(w, [D, D], BF16, isOutput=False)
    wa0 = nc.declare_dram_parameter("wa0", [D, 1], BF16, isOutput=False)
    wa1 = nc.declare_dram_parameter("wa1", [D, 1], BF16, isOutput=False)
    ident = nc.declare_dram_parameter("ident", [128, 128], BF16, isOutput=False)
    bkq = nc.declare_dram_parameter("bkq", [128, 2], F32, isOutput=False)  # [bk|bq]
    out_d = nc.declare_dram_parameter("out", [NB, D], F32, isOutput=True)

    with tile.TileContext(nc) as tc:
        with (
            tc.tile_pool(name="const", bufs=1) as cpool,
            tc.tile_pool(name="ring", bufs=2) as ring,
            tc.tile_pool(name="big", bufs=1) as big,
            tc.tile_pool(name="ent", bufs=1) as ent,
            tc.tile_pool(name="pk", bufs=2, space="PSUM") as pk,
            tc.tile_pool(name="psc", bufs=2, space="PSUM") as psc,
            tc.tile_pool(name="pv", bufs=2, space="PSUM") as pv,
            tc.tile_pool(name="pm", bufs=2, space="PSUM") as pm,
            tc.tile_pool(name="dram", bufs=1, space="DRAM") as dpool,
        ):
            # ---- constants ----
            w_sb = {}
            for w in ("wk0", "wk1", "wv0", "wv1", "wq0", "wq1"):
                w_sb[w] = cpool.tile([D, D], BF16, tag=w, name=w)
                nc.sync.dma_start(out=w_sb[w][:], in_=wts[w][:])
            wa0_sb = cpool.tile([D, 1], BF16, tag="wa0")
            wa1_sb = cpool.tile([D, 1], BF16, tag="wa1")
            nc.sync.dma_start(out=wa0_sb[:], in_=wa0[:])
            nc.sync.dma_start(out=wa1_sb[:], in_=wa1[:])
            id_sb = cpool.tile([128, 128], BF16, tag="ident")
            nc.sync.dma_start(out=id_sb[:], in_=ident[:])
            bkq_sb = cpool.tile([128, 2], F32, tag="bkq")
            nc.sync.dma_start(out=bkq_sb[:], in_=bkq[:])
            ii_sb = cpool.tile([128, NT], I32, tag="idxi")
            ip_sb = cpool.tile([128, NT], I32, tag="idxp")
            nc.sync.dma_start(out=ii_sb[:], in_=idxi[:])
            nc.sync.dma_start(out=ip_sb[:], in_=idxp[:])
            mb_sb = cpool.tile([NB, P], F32, tag="mb")
            nc.sync.dma_start(out=mb_sb[:], in_=mb[:])

            # ---- gathers + transposes ----
            xeT = big.tile([128, NCOL], BF16, tag="xeT")
            peT = big.tile([128, NCOL], BF16, tag="peT")
            CH = 25           # 25 tiles = 3200 cols = exactly 16 batches
            kT = big.tile([128, NCOL], BF16, tag="kT")
            v_dram = dpool.tile([128, 2 * NB, 128], BF16)
            xe3 = xeT[:].rearrange("p (b t) -> p b t", b=NB)
            pe3 = peT[:].rearrange("p (b t) -> p b t", b=NB)
            for c in range(NT // CH):
                txe = ring.tile([128, CH, 128], BF16, tag="txe", bufs=2)
                tpe = ring.tile([128, CH, 128], BF16, tag="tpe", bufs=2)
                for jj in range(CH):
                    j = c * CH + jj
                    nc.gpsimd.indirect_dma_start(
                        out=txe[:, jj, :], out_offset=None, in_=tbl[:],
                        in_offset=bass.IndirectOffsetOnAxis(ap=ii_sb[:, j:j + 1], axis=0))
                    nc.gpsimd.indirect_dma_start(
                        out=tpe[:, jj, :], out_offset=None, in_=ptbl[:],
                        in_offset=bass.IndirectOffsetOnAxis(ap=ip_sb[:, j:j + 1], axis=0))
                for jj in range(CH):
                    j = c * CH + jj
                    tpx = pm.tile([128, 128], BF16, tag="pmsmall", name="tpx")
                    nc.tensor.transpose(tpx[:], txe[:, jj, :], id_sb[:])
                    nc.vector.tensor_copy(out=xeT[:, j * 128:(j + 1) * 128], in_=tpx[:])
                    tpp = pm.tile([128, 128], BF16, tag="pmsmall", name="tpp")
                    nc.tensor.transpose(tpp[:], tpe[:, jj, :], id_sb[:])
                    nc.scalar.activation(peT[:, j * 128:(j + 1) * 128], tpp[:], Act.Copy)

                b0 = 16 * c
                xc = xe3[:, b0:b0 + 16, :]
                s1 = ring.tile([128, 16, 100], BF16, tag="s1", bufs=1)
                nc.vector.tensor_tensor(out=s1[:, :, 0:99], in0=xc[:, :, 0:99],
                                        in1=xc[:, :, 99:198], op=AluOp.add)
                nc.vector.tensor_copy(out=s1[:, :, 99:100], in_=xc[:, :, 198:199])
                n = 100
                while n > 1:
                    h = n // 2
                    nc.vector.tensor_tensor(out=s1[:, :, 0:h], in0=s1[:, :, 0:h],
                                            in1=s1[:, :, h:2 * h], op=AluOp.add)
                    if n % 2:
                        nc.vector.tensor_copy(out=s1[:, :, h:h + 1],
                                              in_=s1[:, :, n - 1:n])
                        n = h + 1
                    else:
                        n = h
                nc.vector.tensor_scalar(out=xc[:, :, 199:200], in0=s1[:, :, 0:1],
                                        scalar1=1.0 / L, scalar2=None,
                                        op0=AluOp.mult)

                for g in range(8):
                    cols = slice(3200 * c + 400 * g, 3200 * c + 400 * (g + 1))
                    kps = pk.tile([128, 400], F32, tag="kps")
                    nc.tensor.matmul(kps[:], w_sb["wk0"][:], xeT[:, cols],
                                     start=True, stop=False)
                    nc.tensor.matmul(kps[:], w_sb["wk1"][:], peT[:, cols],
                                     start=False, stop=True)
                    nc.scalar.activation(kT[:, cols], kps[:], Act.Relu,
                                         bias=bkq_sb[:, 0:1])

                for pr in range(8):
                    vps = pv.tile([128, 512], F32, tag="vps")
                    for h in range(2):
                        b = b0 + 2 * pr + h
                        cA = slice(b * P, b * P + 128)
                        cB = slice(b * P + 128, (b + 1) * P)
                        oA = vps[:, 256 * h:256 * h + 128]
                        nc.tensor.matmul(oA, xeT[:, cA], w_sb["wv0"][:],
                                         start=True, stop=False)
                        nc.tensor.matmul(oA, peT[:, cA], w_sb["wv1"][:],
                                         start=False, stop=True)
                        oB = vps[0:72, 256 * h + 128:256 * h + 256]
                        nc.tensor.matmul(oB, xeT[:, cB], w_sb["wv0"][:],
                                         start=True, stop=False)
                        nc.tensor.matmul(oB, peT[:, cB], w_sb["wv1"][:],
                                         start=False, stop=True)
                    v4 = vps[:].rearrange("p (t d) -> p t d", d=128)
                    vstg = ring.tile([128, 4, 128], BF16, tag="vstg", bufs=2)
                    nc.vector.memset(vstg[64:128, 1::2, :], 0.0)
                    nc.scalar.activation(vstg[:, 0::2, :], v4[:, 0::2, :], Act.Relu)
                    nc.scalar.activation(vstg[0:72, 1::2, :], v4[0:72, 1::2, :],
                                         Act.Relu)
                    nc.sync.dma_start(
                        out=v_dram[:, 4 * (8 * c + pr):4 * (8 * c + pr) + 4, :],
                        in_=vstg[:])

            # ---- q (last token only) and alpha ----
            xl0 = xe3[:, :, 199]      # [128 dim, 128 b] strided view
            xl1 = pe3[:, :, 199]
            qa_ps = pm.tile([128, 128], F32, tag="pmsmall")
            nc.tensor.matmul(qa_ps[:], w_sb["wq0"][:], xl0, start=True, stop=False)
            nc.tensor.matmul(qa_ps[:], w_sb["wq1"][:], xl1, start=False, stop=True)
            qT = ent.tile([128, 128], BF16, tag="qT")
            # q = relu(z + bq) * (1/sqrt(D)); bq broadcast per-partition (dout)
            nc.scalar.activation(qT[:], qa_ps[:], Act.Relu,
                                 bias=bkq_sb[:, 1:2], scale=1.0)
            nc.vector.tensor_scalar(out=qT[:], in0=qT[:],
                                    scalar1=1.0 / math.sqrt(D), scalar2=None,
                                    op0=AluOp.mult)

            al_ps = pm.tile([128, 1], F32, tag="pmsmall")
            nc.tensor.matmul(al_ps[:], xl0, wa0_sb[:], start=True, stop=False)
            nc.tensor.matmul(al_ps[:], xl1, wa1_sb[:], start=False, stop=True)
            am1 = ent.tile([128, 1], F32, tag="am1")        # alpha-1 = sigmoid(.+ba)
            nc.scalar.activation(am1[:], al_ps[:], Act.Sigmoid, bias=ba_const)
            cexp = ent.tile([128, 1], F32, tag="cexp")      # 1/(alpha-1)
            nc.vector.reciprocal(cexp[:], am1[:])
            thi_off = ent.tile([128, 1], F32, tag="thi")    # (1/P)^(alpha-1)
            nc.scalar.activation(thi_off[:], am1[:], Act.Exp, scale=-math.log(P))

            # ---- scores: per-batch M=1 matmuls (partition 0), staged evac ----
            scores = ent.tile([NB, P], F32, tag="scores")
            for chunk in range(16):                     # 8 batches per chunk
                stg = ent.tile([1, 8 * P], F32, tag="stg", bufs=1)
                for kb in range(4):                     # 2 batches per bank
                    sp = psc.tile([128, 512], F32, tag="scps")
                    for sl in range(2):
                        b = 8 * chunk + 2 * kb + sl
                        nc.tensor.matmul(
                            sp[0:1, 256 * sl:256 * sl + 200],
                            qT[:, b:b + 1],
                            kT[:, b * P:(b + 1) * P],
                            start=True, stop=True)
                    st3 = stg[:].rearrange("p (b t) -> p b t", b=8)
                    nc.scalar.activation(
                        st3[:, 2 * kb:2 * kb + 2, :],
                        sp[0:1, :].rearrange("p (s f) -> p s f", s=2)[:, :, 0:200],
                        Act.Copy)
                nc.sync.dma_start(
                    out=scores[8 * chunk:8 * chunk + 8, :],
                    in_=stg[:].rearrange("p (b t) -> p b t", b=8))

            # ---- entmax bisection (batch-major [128, 200]) ----
            nc.vector.tensor_tensor(out=scores[:], in0=scores[:], in1=mb_sb[:],
                                    op=AluOp.add)
            Xa = ent.tile([NB, P], F32, tag="Xa")
            nc.vector.tensor_scalar(out=Xa[:], in0=scores[:], scalar1=am1[:],
                                    scalar2=None, op0=AluOp.mult)
            mx = ent.tile([NB, 1], F32, tag="mx")
            nc.vector.tensor_reduce(mx[:], Xa[:], axis=mybir.AxisListType.X,
                                    op=AluOp.max)
            tlo = ent.tile([NB, 1], F32, tag="tlo")
            nc.vector.tensor_scalar(out=tlo[:], in0=mx[:], scalar1=-1.0,
                                    scalar2=None, op0=AluOp.add)
            dm = ent.tile([NB, 1], F32, tag="dm")   # tau_hi - tau_lo = 1 - thi_off
            nc.vector.tensor_scalar(out=dm[:], in0=thi_off[:], scalar1=-1.0,
                                    scalar2=-1.0, op0=AluOp.mult, op1=AluOp.subtract)
            # dm = thi_off*-1 - (-1) = 1 - thi_off
            tm = ent.tile([NB, 1], F32, tag="tm")
            z = ent.tile([NB, P], F32, tag="z")
            e = ent.tile([NB, P], F32, tag="e")
            S = ent.tile([NB, 1], F32, tag="S")
            msk = ent.tile([NB, 1], I32, tag="msk")
            for it in range(N_ITER):
                nc.vector.tensor_scalar(out=dm[:], in0=dm[:], scalar1=0.5,
                                        scalar2=None, op0=AluOp.mult)
                nc.vector.tensor_tensor(out=tm[:], in0=tlo[:], in1=dm[:],
                                        op=AluOp.add)
                nc.vector.tensor_scalar(out=z[:], in0=Xa[:], scalar1=tm[:],
                                        scalar2=1e-30, op0=AluOp.subtract,
                                        op1=AluOp.max)
                nc.scalar.activation(z[:], z[:], Act.Ln)
                nc.scalar.activation(e[:], z[:], Act.Exp, scale=cexp[:],
                                     accum_out=S[:])
                nc.vector.tensor_scalar(out=msk[:], in0=S[:], scalar1=1.0,
                                        scalar2=None, op0=AluOp.is_ge)
                nc.vector.copy_predicated(out=tlo[:], mask=msk[:], data=tm[:])
            attw = ent.tile([NB, P], BF16, tag="attw")
            nc.vector.reciprocal(S[:], S[:])
            nc.vector.tensor_scalar(out=attw[:], in0=e[:], scalar1=S[:],
                                    scalar2=None, op0=AluOp.mult)

            # ---- attw^T (token-major) via PE transposes ----
            attwT = ent.tile([128, 2, 128], BF16, tag="attwT")
            nc.vector.memset(attwT[:], 0.0)
            t0 = pm.tile([128, 128], BF16, tag="pmsmall")
            nc.tensor.transpose(t0[:], attw[:, 0:128], id_sb[:])
            nc.vector.tensor_copy(out=attwT[:, 0, :], in_=t0[:])
            t1 = pm.tile([72, 128], BF16, tag="pmsmall")
            nc.tensor.transpose(t1[:], attw[:, 128:200], id_sb[:])
            nc.vector.tensor_copy(out=attwT[0:72, 1, :], in_=t1[:])

            # ---- AV -> att^T [d, b] ----
            attT_ps = pm.tile([128, 128], F32, tag="pmsmall")
            for g in range(NB // 8):
                vav = ring.tile([128, 16, 128], BF16, tag="vav", bufs=2)
                nc.sync.dma_start(out=vav[:],
                                  in_=v_dram[:, 16 * g:16 * g + 16, :])
                for bl in range(8):
                    b = 8 * g + bl
                    nc.tensor.matmul(attT_ps[:, b:b + 1], vav[:, 2 * bl, :],
                                     attwT[:, 0, b:b + 1], start=True, stop=False)
                    nc.tensor.matmul(attT_ps[:, b:b + 1], vav[:, 2 * bl + 1, :],
                                     attwT[:, 1, b:b + 1], start=False, stop=True)
            attT_sb = ent.tile([128, 128], BF16, tag="attTs")
            nc.scalar.activation(attT_sb[:], attT_ps[:], Act.Copy)
            att_ps = pm.tile([128, 128], BF16, tag="pmsmall")
            nc.tensor.transpose(att_ps[:], attT_sb[:], id_sb[:])
            attR = ent.tile([NB, D], F32, tag="attR")
            nc.scalar.activation(attR[:], att_ps[:], Act.Relu)

            # ---- L2 normalize ----
            sq = ent.tile([NB, D], F32, tag="sq")
            s2 = ent.tile([NB, 1], F32, tag="s2")
            nc.scalar.activation(sq[:], attR[:], Act.Square)
            nc.vector.tensor_reduce(s2[:], sq[:], axis=mybir.AxisListType.X,
                                    op=AluOp.add)
            nc.scalar.activation(s2[:], s2[:], Act.Sqrt)
            nc.vector.tensor_scalar(out=s2[:], in0=s2[:], scalar1=1e-12,
                                    scalar2=None, op0=AluOp.max)
            nc.vector.reciprocal(s2[:], s2[:])
            out_sb = ent.tile([NB, D], F32, tag="out")
            nc.vector.tensor_scalar(out=out_sb[:], in0=attR[:], scalar1=s2[:],
                                    scalar2=None, op0=AluOp.mult)
            nc.sync.dma_start(out=out_d[:], in_=out_sb[:])

    nc.compile()
    return nc


def _prep_core(c, x, pos, item_bf, pos_bf):
    """Host-side per-core staging: compacted table + index buffers + mask."""
    xs = x[c * NB:(c + 1) * NB].astype(np.int64)          # [128, 199]
    ps = pos[c * NB:(c + 1) * NB].astype(np.int64)        # [128, 200]
    mask0 = xs == 0
    xi = np.where(mask0, V, xs)
    uniq, inv = np.unique(xi, return_inverse=True)
    inv = inv.reshape(xs.shape)
    if uniq[-1] != V:
        uniq = np.append(uniq, V)
    z_id = len(uniq) - 1 if uniq[-1] == V else int(np.searchsorted(uniq, V))
    z_id = int(np.where(uniq == V)[0][0])
    tbl = np.zeros((TBL_ROWS, D), dtype=ml_dtypes.bfloat16)
    tbl[:len(uniq)] = item_bf[uniq]                       # V row is zeros already

    flat_idx = np.full((NB, P), z_id, dtype=np.int32)
    flat_idx[:, :L] = inv
    flat_idx = flat_idx.reshape(-1)                        # [25600] flat=200b+t
    idxi = flat_idx.reshape(NT, 128).T.copy()              # idxi[p,j]=flat[128j+p]

    pflat = ps.reshape(-1).astype(np.int32)
    idxp = pflat.reshape(NT, 128).T.copy()

    mb = np.zeros((NB, P), dtype=np.float32)
    mb[:, :L] = np.where(mask0, -1e30, 0.0)
    return {"tbl": tbl, "idxi": idxi, "idxp": idxp, "mb": mb}


def kernel(x, pos, item_emb, pos_emb, Wq, bq, Wk, bk, Wv, bv, wa, ba):
    x = np.asarray(x)
    pos = np.asarray(pos)
    item_emb = np.asarray(item_emb, dtype=np.float32)
    pos_emb = np.asarray(pos_emb, dtype=np.float32)

    item_bf = np.vstack([item_emb, np.zeros((1, D), np.float32)]).astype(
        ml_dtypes.bfloat16)
    pos_bf = np.asarray(pos_emb, dtype=ml_dtypes.bfloat16)

    wb = {}
    for name, W in (("wk", Wk), ("wv", Wv), ("wq", Wq)):
        W = np.asarray(W, np.float32)
        wb[name + "0"] = W[:D].astype(ml_dtypes.bfloat16)
        wb[name + "1"] = W[D:].astype(ml_dtypes.bfloat16)
    wa = np.asarray(wa, np.float32)
    wa0 = wa[:D].astype(ml_dtypes.bfloat16)
    wa1 = wa[D:].astype(ml_dtypes.bfloat16)
    bkq = np.stack([np.asarray(bk, np.float32),
                    np.asarray(bq, np.float32)], axis=1)   # [128, 2]
    ba_const = float(np.asarray(ba, np.float32).reshape(-1)[0])
    ident = np.eye(128, dtype=ml_dtypes.bfloat16)

    key = ("k", ba_const)
    if key not in _cache:
        _cache[key] = _build(ba_const)
    nc = _cache[key]

    shared = {"ptbl": pos_bf, "wa0": wa0, "wa1": wa1, "ident": ident, "bkq": bkq}
    shared.update({k: wb[k] for k in wb})
    in_maps = []
    for c in range(NCORES):
        m = dict(shared)
        m.update(_prep_core(c, x, pos, item_bf, pos_bf))
        in_maps.append(m)

    global _last_in_maps
    _last_in_maps = in_maps
    res = run_bass_kernel_spmd(nc, in_maps, core_ids=list(range(NCORES)))
    out = np.concatenate([res.results[c]["out"] for c in range(NCORES)], axis=0)
    return out.astype(np.float32)


if __name__ == "__main__":
    d = np.load('/tmp/inputs.npz')
    inp = {k: d[k] for k in d.files}
    got = kernel(**inp)
    ref = np.load('/tmp/ref_out.npy')
    err = np.abs(got - ref).max() / np.abs(ref).max()
    fro = np.linalg.norm(got - ref) / np.linalg.norm(ref)
    print(f"max_rel={err:.3e} fro_rel={fro:.3e}")



# revision 7
# speedup vs baseline: 2.9682x; 2.9682x over previous
"""DualAttention Trainium2 Bass kernel (8-core data-parallel), v2.

Contract: kernel(**inputs) takes the FULL inputs of nn_DualAttention
(B=1024, L=199, V=50000, D=Dp=128) and returns the full [1024, 128] f32
output, equal to reference.reference(**inputs).

v2 strategy (per core, 128 batch rows; only the LAST attention row is
needed):
 - embeddings staged feature-major per core by the host (the on-device
   batched-gather primitives -- InstDMAGatherAnt and multi-column
   indirect DMA offsets -- are broken in this environment; 400 serial
   single-column SWDGE gathers cost ~450us and dominate, so the gather
   is done host-side and the device streams linear chunks).
 - per chunk: masked mean (DVE reduce), K projection (weights
   stationary), q projection, token-major V (data stationary, zero-padded
   chunk tails for uniform 128-wide tiles), scores computed TOKEN-major
   (stationary kT tiles x moving q column) accumulated in one PSUM tile.
 - V kept in SBUF (no DRAM roundtrip).
 - tail: alpha, f32 PE transposes of scoresT, entmax bisection, attw
   transposes, AV (stationary v tiles), relu + L2-norm.
"""
import sys
sys.path.insert(0, '/opt/trn_rl_repo')

import math
import numpy as np
import ml_dtypes

import concourse.bass as bass
import concourse.bacc as bacc
import concourse.mybir as mybir
import concourse.tile as tile
from concourse.bass_utils import run_bass_kernel_spmd

F32 = mybir.dt.float32
BF16 = mybir.dt.bfloat16
I16 = mybir.dt.int16

B, L, V, D = 1024, 199, 50000, 128
P = L + 1                  # 200 tokens (199 items + mean)
NB = 128                   # batches per core
NCORES = 8
NCHUNK = 8
CB = NB // NCHUNK          # 16 batches per chunk
CCOL = CB * P              # 3200 gathered columns per chunk
CPAD = CCOL + 128          # chunk cols + zero tail (tileB overhang)
KCOL = NCHUNK * CPAD       # 26624 kT columns (per-chunk zero pad)
TBL_ROWS = 25728           # fixed-size per-core compact item table
N_ITER = 26                # bisection iterations
AluOp = mybir.AluOpType
Act = mybir.ActivationFunctionType

_cache = {}
_last_in_maps = None


def _build(ba_const: float):
    nc = bacc.Bacc(None, target_bir_lowering=False, debug=False)

    xeT_d = nc.declare_dram_parameter("xeT", [128, NB * P], BF16, isOutput=False)
    peT_d = nc.declare_dram_parameter("peT", [128, NB * P], BF16, isOutput=False)
    mb = nc.declare_dram_parameter("mb", [NB, P], F32, isOutput=False)
    wts = {}
    for w in ("wk0", "wk1", "wv0", "wv1", "wq0", "wq1"):
        wts[w] = nc.declare_dram_parameter(w, [D, D], BF16, isOutput=False)
    wa0 = nc.declare_dram_parameter("wa0", [D, 1], BF16, isOutput=False)
    wa1 = nc.declare_dram_parameter("wa1", [D, 1], BF16, isOutput=False)
    ident = nc.declare_dram_parameter("ident", [128, 128], BF16, isOutput=False)
    identf = nc.declare_dram_parameter("identf", [128, 128], F32, isOutput=False)
    bkq = nc.declare_dram_parameter("bkq", [128, 2], F32, isOutput=False)
    out_d = nc.declare_dram_parameter("out", [NB, D], F32, isOutput=True)

    with tile.TileContext(nc) as tc:
        with (
            tc.tile_pool(name="const", bufs=1) as cpool,
            tc.tile_pool(name="ring", bufs=2) as ring,
            tc.tile_pool(name="big", bufs=1) as big,
            tc.tile_pool(name="ent", bufs=1) as ent,
            tc.tile_pool(name="pk", bufs=2, space="PSUM") as pk,
            tc.tile_pool(name="pv", bufs=2, space="PSUM") as pv,
            tc.tile_pool(name="psc", bufs=1, space="PSUM") as psc,
            tc.tile_pool(name="pq", bufs=1, space="PSUM") as pq,
            tc.tile_pool(name="pm", bufs=2, space="PSUM") as pm,
        ):
            # ---- constants ----
            w_sb = {}
            for w in ("wk0", "wk1", "wv0", "wv1", "wq0", "wq1"):
                w_sb[w] = cpool.tile([D, D], BF16, tag=w, name=w)
                nc.sync.dma_start(out=w_sb[w][:], in_=wts[w][:])
            wa0_sb = cpool.tile([D, 1], BF16, tag="wa0")
            wa1_sb = cpool.tile([D, 1], BF16, tag="wa1")
            nc.sync.dma_start(out=wa0_sb[:], in_=wa0[:])
            nc.sync.dma_start(out=wa1_sb[:], in_=wa1[:])
            id_sb = cpool.tile([128, 128], BF16, tag="ident")
            nc.sync.dma_start(out=id_sb[:], in_=ident[:])
            idf_sb = cpool.tile([128, 128], F32, tag="identf")
            nc.sync.dma_start(out=idf_sb[:], in_=identf[:])
            bkq_sb = cpool.tile([128, 2], F32, tag="bkq")
            nc.sync.dma_start(out=bkq_sb[:], in_=bkq[:])
            mb_sb = cpool.tile([NB, P], F32, tag="mb")
            nc.sync.dma_start(out=mb_sb[:], in_=mb[:])

            kT = big.tile([128, KCOL], BF16, tag="kT")
            v_sb = big.tile([128, 2 * NB, 128], BF16, tag="v")
            qT = ent.tile([128, NB], BF16, tag="qT")
            qlx = ent.tile([128, NB], BF16, tag="qlx")
            qlp = ent.tile([128, NB], BF16, tag="qlp")
            # scoresT accumulator: [token-tile 2][token 128][batch 128]
            sT_ps = psc.tile([128, 2, NB], F32, tag="sT")

            for c in range(NCHUNK):
                gx = ring.tile([128, CPAD], BF16, tag="gx", bufs=2)
                gp = ring.tile([128, CPAD], BF16, tag="gp", bufs=2)
                nc.sync.dma_start(out=gx[:, 0:CCOL],
                                  in_=xeT_d[:, CCOL * c:CCOL * (c + 1)])
                nc.sync.dma_start(out=gp[:, 0:CCOL],
                                  in_=peT_d[:, CCOL * c:CCOL * (c + 1)])
                nc.vector.memset(gx[:, CCOL:CPAD], 0.0)
                nc.vector.memset(gp[:, CCOL:CPAD], 0.0)

                gx3 = gx[:, 0:CCOL].rearrange("p (b t) -> p b t", b=CB)
                gp3 = gp[:, 0:CCOL].rearrange("p (b t) -> p b t", b=CB)

                # masked mean over the 199 item tokens -> column 199
                mean_f = ring.tile([128, CB], F32, tag="mean", bufs=2)
                nc.vector.tensor_reduce(mean_f[:], gx3[:, :, 0:L],
                                        axis=mybir.AxisListType.X, op=AluOp.add)
                nc.vector.tensor_scalar(out=gx3[:, :, L], in0=mean_f[:],
                                        scalar1=1.0 / L, scalar2=None,
                                        op0=AluOp.mult)

                # stash last-token columns for alpha
                nc.vector.tensor_copy(out=qlx[:, CB * c:CB * (c + 1)],
                                      in_=gx3[:, :, L])
                nc.vector.tensor_copy(out=qlp[:, CB * c:CB * (c + 1)],
                                      in_=gp3[:, :, L])

                # K projection (feature-major, weights stationary)
                for g in range(8):
                    cols = slice(400 * g, 400 * (g + 1))
                    kcols = slice(CPAD * c + 400 * g, CPAD * c + 400 * (g + 1))
                    kps = pk.tile([128, 400], F32, tag="kps")
                    nc.tensor.matmul(kps[:], w_sb["wk0"][:], gx[:, cols],
                                     start=True, stop=False)
                    nc.tensor.matmul(kps[:], w_sb["wk1"][:], gp[:, cols],
                                     start=False, stop=True)
                    nc.scalar.activation(kT[:, kcols], kps[:], Act.Relu,
                                         bias=bkq_sb[:, 0:1])
                nc.vector.memset(kT[:, CPAD * c + CCOL:CPAD * (c + 1)], 0.0)

                # q projection for this chunk's 16 batches (scaled by 1/sqrt(D))
                qps = pq.tile([128, CB], F32, tag="qps")
                nc.tensor.matmul(qps[:], w_sb["wq0"][:], gx3[:, :, L],
                                 start=True, stop=False)
                nc.tensor.matmul(qps[:], w_sb["wq1"][:], gp3[:, :, L],
                                 start=False, stop=True)
                nc.scalar.activation(qT[:, CB * c:CB * (c + 1)], qps[:],
                                     Act.Relu, bias=bkq_sb[:, 1:2],
                                     scale=1.0 / math.sqrt(D))

                # scoresT: per batch, stationary kT tiles x moving q column
                for l in range(CB):
                    gb = CB * c + l
                    base = CPAD * c + P * l
                    nc.tensor.matmul(sT_ps[:, 0, gb:gb + 1],
                                     kT[:, base:base + 128],
                                     qT[:, gb:gb + 1], start=True, stop=True)
                    nc.tensor.matmul(sT_ps[:, 1, gb:gb + 1],
                                     kT[:, base + 128:base + 256],
                                     qT[:, gb:gb + 1], start=True, stop=True)

                # V projection (token-major, data stationary), 2 batches/bank
                for pr in range(CB // 2):
                    vps = pv.tile([128, 4, 128], F32, tag="vps")
                    for h in range(4):
                        l = 2 * pr + h // 2
                        tcols = slice(P * l + 128 * (h % 2),
                                      P * l + 128 * (h % 2) + 128)
                        nc.tensor.matmul(vps[:, h, :], gx[:, tcols],
                                         w_sb["wv0"][:], start=True, stop=False)
                        nc.tensor.matmul(vps[:, h, :], gp[:, tcols],
                                         w_sb["wv1"][:], start=False, stop=True)
                    vbase = 4 * (CB // 2 * c + pr)
                    nc.scalar.activation(v_sb[:, vbase:vbase + 4, :], vps[:],
                                         Act.Relu)

            # ---- alpha ----
            al_ps = pq.tile([128, 1], F32, tag="qps")
            nc.tensor.matmul(al_ps[:], qlx[:], wa0_sb[:], start=True, stop=False)
            nc.tensor.matmul(al_ps[:], qlp[:], wa1_sb[:], start=False, stop=True)
            am1 = ent.tile([128, 1], F32, tag="am1")     # alpha-1 = sigmoid(.+ba)
            nc.scalar.activation(am1[:], al_ps[:], Act.Sigmoid, bias=ba_const)
            cexp = ent.tile([128, 1], F32, tag="cexp")   # 1/(alpha-1)
            nc.vector.reciprocal(cexp[:], am1[:])
            thi_off = ent.tile([128, 1], F32, tag="thi")  # (1/P)^(alpha-1)
            nc.scalar.activation(thi_off[:], am1[:], Act.Exp, scale=-math.log(P))

            # ---- scores -> batch-major via f32 PE transposes ----
            sT_sb = ent.tile([128, 2, NB], F32, tag="sTsb")
            nc.scalar.activation(sT_sb[:], sT_ps[:], Act.Copy)
            scores = ent.tile([NB, P], F32, tag="scores")
            for h in range(2):
                tp = pm.tile([128, 128], F32, tag="pm", name=f"st{h}")
                nc.tensor.transpose(tp[:], sT_sb[:, h, :], idf_sb[:])
                n = 128 if h == 0 else P - 128
                nc.scalar.activation(scores[:, 128 * h:128 * h + n],
                                     tp[:, 0:n], Act.Copy)

            # ---- entmax bisection (batch-major [128, 200]) ----
            Xa = ent.tile([NB, P], F32, tag="Xa")
            nc.vector.tensor_tensor(out=Xa[:], in0=scores[:], in1=mb_sb[:],
                                    op=AluOp.add)
            nc.vector.tensor_scalar(out=Xa[:], in0=Xa[:], scalar1=am1[:],
                                    scalar2=None, op0=AluOp.mult)
            mx = ent.tile([NB, 1], F32, tag="mx")
            nc.vector.tensor_reduce(mx[:], Xa[:], axis=mybir.AxisListType.X,
                                    op=AluOp.max)
            tlo = ent.tile([NB, 1], F32, tag="tlo")
            nc.vector.tensor_scalar(out=tlo[:], in0=mx[:], scalar1=-1.0,
                                    scalar2=None, op0=AluOp.add)
            # dm table: dm_i = (1 - thi_off) * 2^-(i+1)
            dmtab = ent.tile([NB, N_ITER], F32, tag="dmtab")
            nc.vector.tensor_scalar(out=dmtab[:, 0:1], in0=thi_off[:],
                                    scalar1=-0.5, scalar2=0.5,
                                    op0=AluOp.mult, op1=AluOp.add)
            for i in range(1, N_ITER):
                nc.vector.tensor_scalar(out=dmtab[:, i:i + 1],
                                        in0=dmtab[:, i - 1:i], scalar1=0.5,
                                        scalar2=None, op0=AluOp.mult)
            eps = ent.tile([NB, 1], F32, tag="eps")
            nc.vector.memset(eps[:], 1e-30)
            tm = ent.tile([NB, 1], F32, tag="tm")
            ntm = ent.tile([NB, 1], F32, tag="ntm")
            z = ent.tile([NB, P], F32, tag="z")
            e = ent.tile([NB, P], F32, tag="e")
            S = ent.tile([NB, 1], F32, tag="S")
            msk = ent.tile([NB, 1], mybir.dt.int32, tag="msk")
            for it in range(N_ITER):
                nc.vector.tensor_scalar(out=tm[:], in0=tlo[:],
                                        scalar1=dmtab[:, it:it + 1],
                                        scalar2=None, op0=AluOp.add)
                nc.vector.tensor_scalar(out=ntm[:], in0=tm[:], scalar1=-1.0,
                                        scalar2=None, op0=AluOp.mult)
                nc.scalar.activation(z[:], Xa[:], Act.Relu, bias=ntm[:])
                nc.scalar.activation(z[:], z[:], Act.Ln, bias=eps[:])
                nc.scalar.activation(e[:], z[:], Act.Exp, scale=cexp[:],
                                     accum_out=S[:])
                nc.vector.tensor_scalar(out=msk[:], in0=S[:], scalar1=1.0,
                                        scalar2=None, op0=AluOp.is_ge)
                nc.vector.copy_predicated(out=tlo[:], mask=msk[:], data=tm[:])
            attw = ent.tile([NB, P], BF16, tag="attw")
            nc.vector.reciprocal(S[:], S[:])
            nc.vector.tensor_scalar(out=attw[:], in0=e[:], scalar1=S[:],
                                    scalar2=None, op0=AluOp.mult)

            # ---- attw^T (token-major) via PE transposes ----
            attwT = ent.tile([128, 2, 128], BF16, tag="attwT")
            nc.vector.memset(attwT[64:128, 1, :], 0.0)
            t0 = pm.tile([128, 128], BF16, tag="pm", name="t0")
            nc.tensor.transpose(t0[:], attw[:, 0:128], id_sb[:])
            nc.vector.tensor_copy(out=attwT[:, 0, :], in_=t0[:])
            t1 = pm.tile([72, 128], BF16, tag="pm", name="t1")
            nc.tensor.transpose(t1[:], attw[:, 128:200], id_sb[:])
            nc.vector.tensor_copy(out=attwT[0:72, 1, :], in_=t1[:])

            # ---- AV -> att^T [d, b] ----
            attT_ps = pm.tile([128, 128], F32, tag="pm", name="avps")
            for b in range(NB):
                nc.tensor.matmul(attT_ps[:, b:b + 1], v_sb[:, 2 * b, :],
                                 attwT[:, 0, b:b + 1], start=True, stop=False)
                nc.tensor.matmul(attT_ps[:, b:b + 1], v_sb[:, 2 * b + 1, :],
                                 attwT[:, 1, b:b + 1], start=False, stop=True)
            attT_sb = ent.tile([128, 128], BF16, tag="attTs")
            nc.scalar.activation(attT_sb[:], attT_ps[:], Act.Copy)
            att_ps = pm.tile([128, 128], BF16, tag="pm", name="attps")
            nc.tensor.transpose(att_ps[:], attT_sb[:], id_sb[:])
            attR = ent.tile([NB, D], F32, tag="attR")
            nc.scalar.activation(attR[:], att_ps[:], Act.Relu)

            # ---- L2 normalize ----
            sq = ent.tile([NB, D], F32, tag="sq")
            s2 = ent.tile([NB, 1], F32, tag="s2")
            nc.scalar.activation(sq[:], attR[:], Act.Square, accum_out=s2[:])
            nc.scalar.activation(s2[:], s2[:], Act.Sqrt)
            nc.vector.tensor_scalar(out=s2[:], in0=s2[:], scalar1=1e-12,
                                    scalar2=None, op0=AluOp.max)
            nc.vector.reciprocal(s2[:], s2[:])
            out_sb = ent.tile([NB, D], F32, tag="out")
            nc.vector.tensor_scalar(out=out_sb[:], in0=attR[:], scalar1=s2[:],
                                    scalar2=None, op0=AluOp.mult)
            nc.sync.dma_start(out=out_d[:], in_=out_sb[:])

    nc.compile()
    return nc


def _prep_core(c, x, pos, item_bf, pos_bf):
    """Host-side per-core staging: feature-major embeddings + mask."""
    xs = x[c * NB:(c + 1) * NB].astype(np.int64)          # [128, 199]
    ps = pos[c * NB:(c + 1) * NB].astype(np.int64)        # [128, 200]
    mask0 = xs == 0
    xi = np.where(mask0, V, xs)                           # zeros row for masked
    flat_idx = np.full((NB, P), V, dtype=np.int64)        # col 199 -> zeros row
    flat_idx[:, :L] = xi
    xeT = np.ascontiguousarray(item_bf[flat_idx.reshape(-1)].T)  # [128, 25600]
    peT = np.ascontiguousarray(pos_bf[ps.reshape(-1)].T)         # [128, 25600]

    mb = np.zeros((NB, P), dtype=np.float32)
    mb[:, :L] = np.where(mask0, -1e30, 0.0)
    return {"xeT": xeT, "peT": peT, "mb": mb}


def kernel(x, pos, item_emb, pos_emb, Wq, bq, Wk, bk, Wv, bv, wa, ba):
    x = np.asarray(x)
    pos = np.asarray(pos)
    item_emb = np.asarray(item_emb, dtype=np.float32)
    pos_emb = np.asarray(pos_emb, dtype=np.float32)

    item_bf = np.vstack([item_emb, np.zeros((1, D), np.float32)]).astype(
        ml_dtypes.bfloat16)
    pos_bf = pos_emb.astype(ml_dtypes.bfloat16)

    wb = {}
    for name, W in (("wk", Wk), ("wv", Wv), ("wq", Wq)):
        W = np.asarray(W, np.float32)
        wb[name + "0"] = W[:D].astype(ml_dtypes.bfloat16)
        wb[name + "1"] = W[D:].astype(ml_dtypes.bfloat16)
    wa = np.asarray(wa, np.float32)
    wa0 = wa[:D].astype(ml_dtypes.bfloat16)
    wa1 = wa[D:].astype(ml_dtypes.bfloat16)
    bkq = np.stack([np.asarray(bk, np.float32),
                    np.asarray(bq, np.float32) / math.sqrt(D)], axis=1)
    ba_const = float(np.asarray(ba, np.float32).reshape(-1)[0])
    ident = np.eye(128, dtype=ml_dtypes.bfloat16)
    identf = np.eye(128, dtype=np.float32)

    key = ("k3", ba_const)
    if key not in _cache:
        _cache[key] = _build(ba_const)
    nc = _cache[key]

    shared = {"wa0": wa0, "wa1": wa1, "ident": ident,
              "identf": identf, "bkq": bkq}
    shared.update(wb)
    in_maps = []
    for c in range(NCORES):
        m = dict(shared)
        m.update(_prep_core(c, x, pos, item_bf, pos_bf))
        in_maps.append(m)

    global _last_in_maps
    _last_in_maps = in_maps
    res = run_bass_kernel_spmd(nc, in_maps, core_ids=list(range(NCORES)))
    out = np.concatenate([res.results[c]["out"] for c in range(NCORES)], axis=0)
    return out.astype(np.float32)


if __name__ == "__main__":
    d = np.load('/tmp/inputs.npz')
    inp = {k: d[k] for k in d.files}
    got = kernel(**inp)
    ref = np.load('/tmp/ref_out.npy')
    err = np.abs(got - ref).max() / np.abs(ref).max()
    fro = np.linalg.norm(got - ref) / np.linalg.norm(ref)
    print(f"max_rel={err:.3e} fro_rel={fro:.3e}")


# revision 8
# speedup vs baseline: 5.1822x; 1.7459x over previous
"""DualAttention Trainium2 Bass kernel (8-core data-parallel), v2.

Contract: kernel(**inputs) takes the FULL inputs of nn_DualAttention
(B=1024, L=199, V=50000, D=Dp=128) and returns the full [1024, 128] f32
output, equal to reference.reference(**inputs).

v2 strategy (per core, 128 batch rows; only the LAST attention row is
needed):
 - embeddings staged feature-major per core by the host (the on-device
   batched-gather primitives -- InstDMAGatherAnt and multi-column
   indirect DMA offsets -- are broken in this environment; 400 serial
   single-column SWDGE gathers cost ~450us and dominate, so the gather
   is done host-side and the device streams linear chunks).
 - per chunk: masked mean (DVE reduce), K projection (weights
   stationary), q projection, token-major V (data stationary, zero-padded
   chunk tails for uniform 128-wide tiles), scores computed TOKEN-major
   (stationary kT tiles x moving q column) accumulated in one PSUM tile.
 - V kept in SBUF (no DRAM roundtrip).
 - tail: alpha, f32 PE transposes of scoresT, entmax bisection, attw
   transposes, AV (stationary v tiles), relu + L2-norm.
"""
import sys
sys.path.insert(0, '/opt/trn_rl_repo')

import math
import numpy as np
import ml_dtypes

import concourse.bass as bass
import concourse.bacc as bacc
import concourse.mybir as mybir
import concourse.tile as tile
from concourse.bass_utils import run_bass_kernel_spmd

F32 = mybir.dt.float32
BF16 = mybir.dt.bfloat16
I16 = mybir.dt.int16

B, L, V, D = 1024, 199, 50000, 128
P = L + 1                  # 200 tokens (199 items + mean)
NB = 128                   # batches per core
NCORES = 8
NCHUNK = 8
CB = NB // NCHUNK          # 16 batches per chunk
CCOL = CB * P              # 3200 gathered columns per chunk
CPAD = CCOL + 128          # chunk cols + zero tail (tileB overhang)
KCOL = NCHUNK * CPAD       # 26624 kT columns (per-chunk zero pad)
TBL_ROWS = 25728           # fixed-size per-core compact item table
N_ITER = 14                # bisection iterations (converged ~1e-6 by 14)
AluOp = mybir.AluOpType
Act = mybir.ActivationFunctionType

_cache = {}
_last_in_maps = None

_COMBINED_SET = "natural_log_exp_and_others"
_OUR_FUNCS = None


def _patched_get_activation_tables(arch):
    """Steer the act-table-load pass to the one set containing every
    function this kernel uses (ln/exp/relu/copy/square), so the ACT engine
    loads its LUT once instead of ping-ponging (~56 loads x 1.3us)."""
    import concourse.hw_specs as _hs
    global _OUR_FUNCS
    if _OUR_FUNCS is None:
        _OUR_FUNCS = {Act.Relu, Act.Ln, Act.Exp, Act.Copy, Act.Square,
                      Act.Identity}
    t = _hs.get_activation_tables(arch)
    out = {}
    for name, fns in t.items():
        out[name] = fns if name == _COMBINED_SET else (fns - _OUR_FUNCS)
    return out


def _build(ba_const: float):
    nc = bacc.Bacc(None, target_bir_lowering=False, debug=False)

    xeT_d = nc.declare_dram_parameter("xeT", [128, NB * P], BF16, isOutput=False)
    peT_d = nc.declare_dram_parameter("peT", [128, NB * P], BF16, isOutput=False)
    mb = nc.declare_dram_parameter("mb", [NB, P], F32, isOutput=False)
    wts = {}
    for w in ("wk0", "wk1", "wv0", "wv1", "wq0", "wq1"):
        wts[w] = nc.declare_dram_parameter(w, [D, D], BF16, isOutput=False)
    wa0 = nc.declare_dram_parameter("wa0", [D, 1], BF16, isOutput=False)
    wa1 = nc.declare_dram_parameter("wa1", [D, 1], BF16, isOutput=False)
    ident = nc.declare_dram_parameter("ident", [128, 128], BF16, isOutput=False)
    identf = nc.declare_dram_parameter("identf", [128, 128], F32, isOutput=False)
    bkq = nc.declare_dram_parameter("bkq", [128, 2], F32, isOutput=False)
    out_d = nc.declare_dram_parameter("out", [NB, D], F32, isOutput=True)

    with tile.TileContext(nc) as tc:
        with (
            tc.tile_pool(name="const", bufs=1) as cpool,
            tc.tile_pool(name="ring", bufs=2) as ring,
            tc.tile_pool(name="big", bufs=1) as big,
            tc.tile_pool(name="ent", bufs=1) as ent,
            tc.tile_pool(name="pk", bufs=2, space="PSUM") as pk,
            tc.tile_pool(name="pv", bufs=2, space="PSUM") as pv,
            tc.tile_pool(name="psc", bufs=1, space="PSUM") as psc,
            tc.tile_pool(name="pq", bufs=1, space="PSUM") as pq,
            tc.tile_pool(name="pm", bufs=2, space="PSUM") as pm,
        ):
            # ---- constants ----
            w_sb = {}
            for w in ("wk0", "wk1", "wv0", "wv1", "wq0", "wq1"):
                w_sb[w] = cpool.tile([D, D], BF16, tag=w, name=w)
                nc.sync.dma_start(out=w_sb[w][:], in_=wts[w][:])
            wa0_sb = cpool.tile([D, 1], BF16, tag="wa0")
            wa1_sb = cpool.tile([D, 1], BF16, tag="wa1")
            nc.sync.dma_start(out=wa0_sb[:], in_=wa0[:])
            nc.sync.dma_start(out=wa1_sb[:], in_=wa1[:])
            id_sb = cpool.tile([128, 128], BF16, tag="ident")
            nc.sync.dma_start(out=id_sb[:], in_=ident[:])
            idf_sb = cpool.tile([128, 128], F32, tag="identf")
            nc.sync.dma_start(out=idf_sb[:], in_=identf[:])
            bkq_sb = cpool.tile([128, 2], F32, tag="bkq")
            nc.sync.dma_start(out=bkq_sb[:], in_=bkq[:])
            mb_sb = cpool.tile([NB, P], F32, tag="mb")
            nc.sync.dma_start(out=mb_sb[:], in_=mb[:])

            kT = big.tile([128, KCOL], BF16, tag="kT")
            v_sb = big.tile([128, 2 * NB, 128], BF16, tag="v")
            qT = ent.tile([128, NB], BF16, tag="qT")
            qlx = ent.tile([128, NB], BF16, tag="qlx")
            qlp = ent.tile([128, NB], BF16, tag="qlp")
            # scoresT accumulator: [token-tile 2][token 128][batch 128]
            sT_ps = psc.tile([128, 2, NB], F32, tag="sT")

            for c in range(NCHUNK):
                gx = ring.tile([128, CPAD], BF16, tag="gx", bufs=2)
                gp = ring.tile([128, CPAD], BF16, tag="gp", bufs=2)
                nc.sync.dma_start(out=gx[:, 0:CCOL],
                                  in_=xeT_d[:, CCOL * c:CCOL * (c + 1)])
                nc.sync.dma_start(out=gp[:, 0:CCOL],
                                  in_=peT_d[:, CCOL * c:CCOL * (c + 1)])
                nc.vector.memset(gx[:, CCOL:CPAD], 0.0)
                nc.vector.memset(gp[:, CCOL:CPAD], 0.0)

                gx3 = gx[:, 0:CCOL].rearrange("p (b t) -> p b t", b=CB)
                gp3 = gp[:, 0:CCOL].rearrange("p (b t) -> p b t", b=CB)

                # masked mean over the 199 item tokens -> column 199
                mean_f = ring.tile([128, CB], F32, tag="mean", bufs=2)
                nc.vector.tensor_reduce(mean_f[:], gx3[:, :, 0:L],
                                        axis=mybir.AxisListType.X, op=AluOp.add)
                nc.vector.tensor_scalar(out=gx3[:, :, L], in0=mean_f[:],
                                        scalar1=1.0 / L, scalar2=None,
                                        op0=AluOp.mult)

                # stash last-token columns for alpha
                nc.vector.tensor_copy(out=qlx[:, CB * c:CB * (c + 1)],
                                      in_=gx3[:, :, L])
                nc.vector.tensor_copy(out=qlp[:, CB * c:CB * (c + 1)],
                                      in_=gp3[:, :, L])

                # K projection (feature-major, weights stationary)
                for g in range(8):
                    cols = slice(400 * g, 400 * (g + 1))
                    kcols = slice(CPAD * c + 400 * g, CPAD * c + 400 * (g + 1))
                    kps = pk.tile([128, 400], F32, tag="kps")
                    nc.tensor.matmul(kps[:], w_sb["wk0"][:], gx[:, cols],
                                     start=True, stop=False)
                    nc.tensor.matmul(kps[:], w_sb["wk1"][:], gp[:, cols],
                                     start=False, stop=True)
                    nc.scalar.activation(kT[:, kcols], kps[:], Act.Relu,
                                         bias=bkq_sb[:, 0:1])
                nc.vector.memset(kT[:, CPAD * c + CCOL:CPAD * (c + 1)], 0.0)

                # q projection for this chunk's 16 batches (scaled by 1/sqrt(D))
                qps = pq.tile([128, CB], F32, tag="qps")
                nc.tensor.matmul(qps[:], w_sb["wq0"][:], gx3[:, :, L],
                                 start=True, stop=False)
                nc.tensor.matmul(qps[:], w_sb["wq1"][:], gp3[:, :, L],
                                 start=False, stop=True)
                nc.scalar.activation(qT[:, CB * c:CB * (c + 1)], qps[:],
                                     Act.Relu, bias=bkq_sb[:, 1:2],
                                     scale=1.0 / math.sqrt(D))

                # scoresT: per batch, stationary kT tiles x moving q column
                for l in range(CB):
                    gb = CB * c + l
                    base = CPAD * c + P * l
                    nc.tensor.matmul(sT_ps[:, 0, gb:gb + 1],
                                     kT[:, base:base + 128],
                                     qT[:, gb:gb + 1], start=True, stop=True)
                    nc.tensor.matmul(sT_ps[:, 1, gb:gb + 1],
                                     kT[:, base + 128:base + 256],
                                     qT[:, gb:gb + 1], start=True, stop=True)

                # V projection (token-major, data stationary), 2 batches/bank
                for pr in range(CB // 2):
                    vps = pv.tile([128, 4, 128], F32, tag="vps")
                    for h in range(4):
                        l = 2 * pr + h // 2
                        tcols = slice(P * l + 128 * (h % 2),
                                      P * l + 128 * (h % 2) + 128)
                        nc.tensor.matmul(vps[:, h, :], gx[:, tcols],
                                         w_sb["wv0"][:], start=True, stop=False)
                        nc.tensor.matmul(vps[:, h, :], gp[:, tcols],
                                         w_sb["wv1"][:], start=False, stop=True)
                    vbase = 4 * (CB // 2 * c + pr)
                    nc.scalar.activation(v_sb[:, vbase:vbase + 4, :], vps[:],
                                         Act.Relu)

            # ---- alpha ----
            al_ps = pq.tile([128, 1], F32, tag="qps")
            nc.tensor.matmul(al_ps[:], qlx[:], wa0_sb[:], start=True, stop=False)
            nc.tensor.matmul(al_ps[:], qlp[:], wa1_sb[:], start=False, stop=True)
            am1 = ent.tile([128, 1], F32, tag="am1")     # alpha-1 = sigmoid(.+ba)
            nc.scalar.activation(am1[:], al_ps[:], Act.Exp, scale=-1.0,
                                 bias=-ba_const)         # e^{-(z+ba)}
            nc.vector.tensor_scalar(out=am1[:], in0=am1[:], scalar1=1.0,
                                    scalar2=None, op0=AluOp.add)
            nc.vector.reciprocal(am1[:], am1[:])
            cexp = ent.tile([128, 1], F32, tag="cexp")   # 1/(alpha-1)
            nc.vector.reciprocal(cexp[:], am1[:])
            thi_off = ent.tile([128, 1], F32, tag="thi")  # (1/P)^(alpha-1)
            nc.scalar.activation(thi_off[:], am1[:], Act.Exp, scale=-math.log(P))

            # ---- scores -> batch-major via f32 PE transposes ----
            sT_sb = ent.tile([128, 2, NB], F32, tag="sTsb")
            nc.scalar.activation(sT_sb[:], sT_ps[:], Act.Copy)
            scores = ent.tile([NB, P], F32, tag="scores")
            for h in range(2):
                tp = pm.tile([128, 128], F32, tag="pm", name=f"st{h}")
                nc.tensor.transpose(tp[:], sT_sb[:, h, :], idf_sb[:])
                n = 128 if h == 0 else P - 128
                nc.scalar.activation(scores[:, 128 * h:128 * h + n],
                                     tp[:, 0:n], Act.Copy)

            # ---- entmax bisection (batch-major [128, 200]) ----
            Xa = ent.tile([NB, P], F32, tag="Xa")
            nc.vector.tensor_tensor(out=Xa[:], in0=scores[:], in1=mb_sb[:],
                                    op=AluOp.add)
            nc.vector.tensor_scalar(out=Xa[:], in0=Xa[:], scalar1=am1[:],
                                    scalar2=None, op0=AluOp.mult)
            mx = ent.tile([NB, 1], F32, tag="mx")
            nc.vector.tensor_reduce(mx[:], Xa[:], axis=mybir.AxisListType.X,
                                    op=AluOp.max)
            tlo = ent.tile([NB, 1], F32, tag="tlo")
            nc.vector.tensor_scalar(out=tlo[:], in0=mx[:], scalar1=-1.0,
                                    scalar2=None, op0=AluOp.add)
            # dm table: dm_i = (1 - thi_off) * 2^-(i+1)
            dmtab = ent.tile([NB, N_ITER], F32, tag="dmtab")
            nc.vector.tensor_scalar(out=dmtab[:, 0:1], in0=thi_off[:],
                                    scalar1=-0.5, scalar2=0.5,
                                    op0=AluOp.mult, op1=AluOp.add)
            for i in range(1, N_ITER):
                nc.vector.tensor_scalar(out=dmtab[:, i:i + 1],
                                        in0=dmtab[:, i - 1:i], scalar1=0.5,
                                        scalar2=None, op0=AluOp.mult)
            eps = ent.tile([NB, 1], F32, tag="eps")
            nc.vector.memset(eps[:], 1e-24)
            tm = ent.tile([NB, 1], F32, tag="tm")
            z = ent.tile([NB, P], F32, tag="z")
            e = ent.tile([NB, P], F32, tag="e")
            S = ent.tile([NB, 1], F32, tag="S")
            msk = ent.tile([NB, 1], mybir.dt.int32, tag="msk")
            for it in range(N_ITER):
                nc.vector.tensor_scalar(out=tm[:], in0=tlo[:],
                                        scalar1=dmtab[:, it:it + 1],
                                        scalar2=None, op0=AluOp.add)
                nc.vector.tensor_scalar(out=z[:], in0=Xa[:], scalar1=tm[:],
                                        scalar2=1e-30, op0=AluOp.subtract,
                                        op1=AluOp.max)
                nc.scalar.activation(z[:], z[:], Act.Ln)
                nc.scalar.activation(e[:], z[:], Act.Exp, scale=cexp[:],
                                     accum_out=S[:])
                nc.vector.tensor_scalar(out=msk[:], in0=S[:], scalar1=1.0,
                                        scalar2=None, op0=AluOp.is_ge)
                nc.vector.copy_predicated(out=tlo[:], mask=msk[:], data=tm[:])
            attw = ent.tile([NB, P], BF16, tag="attw")
            nc.vector.reciprocal(S[:], S[:])
            nc.vector.tensor_scalar(out=attw[:], in0=e[:], scalar1=S[:],
                                    scalar2=None, op0=AluOp.mult)

            # ---- attw^T (token-major) via PE transposes ----
            attwT = ent.tile([128, 2, 128], BF16, tag="attwT")
            nc.vector.memset(attwT[64:128, 1, :], 0.0)
            t0 = pm.tile([128, 128], BF16, tag="pm", name="t0")
            nc.tensor.transpose(t0[:], attw[:, 0:128], id_sb[:])
            nc.vector.tensor_copy(out=attwT[:, 0, :], in_=t0[:])
            t1 = pm.tile([72, 128], BF16, tag="pm", name="t1")
            nc.tensor.transpose(t1[:], attw[:, 128:200], id_sb[:])
            nc.vector.tensor_copy(out=attwT[0:72, 1, :], in_=t1[:])

            # ---- AV -> att^T [d, b] ----
            attT_ps = pm.tile([128, 128], F32, tag="pm", name="avps")
            for b in range(NB):
                nc.tensor.matmul(attT_ps[:, b:b + 1], v_sb[:, 2 * b, :],
                                 attwT[:, 0, b:b + 1], start=True, stop=False)
                nc.tensor.matmul(attT_ps[:, b:b + 1], v_sb[:, 2 * b + 1, :],
                                 attwT[:, 1, b:b + 1], start=False, stop=True)
            attT_sb = ent.tile([128, 128], BF16, tag="attTs")
            nc.scalar.activation(attT_sb[:], attT_ps[:], Act.Copy)
            att_ps = pm.tile([128, 128], BF16, tag="pm", name="attps")
            nc.tensor.transpose(att_ps[:], attT_sb[:], id_sb[:])
            attR = ent.tile([NB, D], F32, tag="attR")
            nc.scalar.activation(attR[:], att_ps[:], Act.Relu)

            # ---- L2 normalize ----
            sq = ent.tile([NB, D], F32, tag="sq")
            s2 = ent.tile([NB, 1], F32, tag="s2")
            nc.scalar.activation(sq[:], attR[:], Act.Square, accum_out=s2[:])
            nc.scalar.activation(s2[:], s2[:], Act.Ln, bias=eps[:])
            nc.scalar.activation(s2[:], s2[:], Act.Exp, scale=-0.5)
            out_sb = ent.tile([NB, D], F32, tag="out")
            nc.vector.tensor_scalar(out=out_sb[:], in0=attR[:], scalar1=s2[:],
                                    scalar2=None, op0=AluOp.mult)
            nc.sync.dma_start(out=out_d[:], in_=out_sb[:])

    import concourse.bacc as _bacc_mod
    _orig = _bacc_mod.get_activation_tables
    _bacc_mod.get_activation_tables = _patched_get_activation_tables
    try:
        nc.compile()
    finally:
        _bacc_mod.get_activation_tables = _orig
    return nc


def _prep_core(c, x, pos, item_bf, pos_bf):
    """Host-side per-core staging: feature-major embeddings + mask."""
    xs = x[c * NB:(c + 1) * NB].astype(np.int64)          # [128, 199]
    ps = pos[c * NB:(c + 1) * NB].astype(np.int64)        # [128, 200]
    mask0 = xs == 0
    xi = np.where(mask0, V, xs)                           # zeros row for masked
    flat_idx = np.full((NB, P), V, dtype=np.int64)        # col 199 -> zeros row
    flat_idx[:, :L] = xi
    xeT = np.ascontiguousarray(item_bf[flat_idx.reshape(-1)].T)  # [128, 25600]
    peT = np.ascontiguousarray(pos_bf[ps.reshape(-1)].T)         # [128, 25600]

    mb = np.zeros((NB, P), dtype=np.float32)
    mb[:, :L] = np.where(mask0, -1e30, 0.0)
    return {"xeT": xeT, "peT": peT, "mb": mb}


def kernel(x, pos, item_emb, pos_emb, Wq, bq, Wk, bk, Wv, bv, wa, ba):
    x = np.asarray(x)
    pos = np.asarray(pos)
    item_emb = np.asarray(item_emb, dtype=np.float32)
    pos_emb = np.asarray(pos_emb, dtype=np.float32)

    item_bf = np.vstack([item_emb, np.zeros((1, D), np.float32)]).astype(
        ml_dtypes.bfloat16)
    pos_bf = pos_emb.astype(ml_dtypes.bfloat16)

    wb = {}
    for name, W in (("wk", Wk), ("wv", Wv), ("wq", Wq)):
        W = np.asarray(W, np.float32)
        wb[name + "0"] = W[:D].astype(ml_dtypes.bfloat16)
        wb[name + "1"] = W[D:].astype(ml_dtypes.bfloat16)
    wa = np.asarray(wa, np.float32)
    wa0 = wa[:D].astype(ml_dtypes.bfloat16)
    wa1 = wa[D:].astype(ml_dtypes.bfloat16)
    bkq = np.stack([np.asarray(bk, np.float32),
                    np.asarray(bq, np.float32) / math.sqrt(D)], axis=1)
    ba_const = float(np.asarray(ba, np.float32).reshape(-1)[0])
    ident = np.eye(128, dtype=ml_dtypes.bfloat16)
    identf = np.eye(128, dtype=np.float32)

    key = ("k4", ba_const)
    if key not in _cache:
        _cache[key] = _build(ba_const)
    nc = _cache[key]

    shared = {"wa0": wa0, "wa1": wa1, "ident": ident,
              "identf": identf, "bkq": bkq}
    shared.update(wb)
    in_maps = []
    for c in range(NCORES):
        m = dict(shared)
        m.update(_prep_core(c, x, pos, item_bf, pos_bf))
        in_maps.append(m)

    global _last_in_maps
    _last_in_maps = in_maps
    res = run_bass_kernel_spmd(nc, in_maps, core_ids=list(range(NCORES)))
    out = np.concatenate([res.results[c]["out"] for c in range(NCORES)], axis=0)
    return out.astype(np.float32)


if __name__ == "__main__":
    d = np.load('/tmp/inputs.npz')
    inp = {k: d[k] for k in d.files}
    got = kernel(**inp)
    ref = np.load('/tmp/ref_out.npy')
    err = np.abs(got - ref).max() / np.abs(ref).max()
    fro = np.linalg.norm(got - ref) / np.linalg.norm(ref)
    print(f"max_rel={err:.3e} fro_rel={fro:.3e}")
